# revision 5
# baseline (speedup 1.0000x reference)
"""Distributed Trainium2 Bass kernel for the AIRS-GAT problem.

Sharding (8 cores): core c -> (batch b = c//2, head-group g = c%2).
Each core handles one batch and 4 of the 8 heads (tensor-parallel heads),
with an AllGather of the per-head attention outputs inside each core pair
before a (redundant) full output projection, per the sharding hint.

Key design points:
  * h is kept TRANSPOSED on-chip: hT[d, i] as 4 tiles of [128, 1024].
    QKV projections then take natural-layout weights as stationary operands.
  * Attention is computed transposed: logitsT[j, i] so softmax reduction
    over keys j is handled by PE matmuls (sum) and the normalization is
    deferred past the attn@V matmul (ones-row augmented V gives the sums).
  * Softmax skips max-subtraction (validated: max|logit| ~ 11.4).
  * Edge bias (B,N,N,E)@(E,H) runs on the TensorEngine in fp8e4m3
    DoubleRow mode: edges are split host-side into hi+lo fp8 components
    (exact to ~1e-3) that ride the two DoubleRow slices of one matmul, so
    each e costs 256 PE cycles instead of 512.  ew is quantized to fp8
    with a per-(layer,head) optimal scale c; the compensation c is folded
    into the exp activation's scale operand and 1/c into qw host-side.
  * Per-(head,i-chunk) softmax normalization is pipelined: reciprocal of
    the augmented-row sums -> gpsimd partition broadcast -> fused
    multiply+cast straight out of PSUM, then DMA into the collective
    staging buffer, all overlapped with the next tile's attention math.
  * Matmuls run in bf16 (full-speed); kb and eb are dropped (softmax
    shift invariance).
"""

import os
import sys

import numpy as np

for _p in ("/opt/trn_rl_repo",):
    if _p not in sys.path:
        sys.path.insert(0, _p)

import concourse.bass as bass
import concourse.bacc as bacc
import concourse.mybir as mybir
import concourse.tile as tile
from concourse.bass_utils import run_bass_kernel_spmd

B, N, IN_DIM, D, H, E, L = 4, 1024, 128, 512, 8, 4, 3
HD = D // H          # 64
HPC = H // 2         # heads per core = 4
DPC = HPC * HD       # head dims per core = 256
LN_EPS = 1e-5
NCORES = 8
REPLICA_GROUPS = [[0, 1], [2, 3], [4, 5], [6, 7]]

F32 = mybir.dt.float32
F32R = mybir.dt.float32r
BF16 = mybir.dt.bfloat16
F8 = mybir.dt.float8e4
U16 = mybir.dt.uint16
U8 = mybir.dt.uint8
AX = mybir.AxisListType
ALU = mybir.AluOpType
ACTF = mybir.ActivationFunctionType
DR = mybir.MatmulPerfMode.DoubleRow


def _bf16_bits(x: np.ndarray) -> np.ndarray:
    """fp32 -> bf16 bits (round to nearest even), as uint16."""
    u = np.ascontiguousarray(x, dtype=np.float32).view(np.uint32)
    r = ((u >> 16) & 1) + np.uint32(0x7FFF)
    return ((u + r) >> 16).astype(np.uint16)


def _f8(x: np.ndarray) -> np.ndarray:
    import ml_dtypes

    return np.ascontiguousarray(x, dtype=np.float32).astype(ml_dtypes.float8_e4m3fn)


def _f8_bits(x: np.ndarray) -> np.ndarray:
    return _f8(x).view(np.uint8)


def _opt_scale(ews: np.ndarray) -> float:
    """Scale c minimizing sum_e (fp8(ew_e/c)*c - ew_e)^2."""
    cs = np.geomspace(0.25, 4.0, 400)
    best, bc = None, 1.0
    for c in cs:
        err = float(((_f8(ews / c).astype(np.float32) * c - ews) ** 2).sum())
        if best is None or err < best:
            best, bc = err, float(c)
    return bc


def _build(zeros: dict) -> bass.Bass:
    """Build the SPMD kernel graph (identical program for all 8 cores)."""
    nc = bacc.Bacc(
        "TRN2", target_bir_lowering=False, debug=False, num_devices=NCORES
    )

    dram = {}

    def din(name, shape, dtype=F32):
        dram[name] = nc.dram_tensor(name, list(shape), dtype, kind="ExternalInput")
        return dram[name]

    din("xT", [IN_DIM, N], U16)
    din("edges_f8", [8, E, 128, 2 * N], U8)        # [jt, e, j_in_tile, (hi|lo) i]
    din("in_w", [IN_DIM, D], U16)
    din("qw", [L, D, DPC], U16)                    # pre-scaled by 1/(sqrt(HD)*c_h)
    din("kw", [L, D, DPC], U16)
    din("vw", [L, D, DPC], U16)
    din("ow", [L, D, D], U16)                      # full out-proj weight
    din("ew8_diag", [L, HPC, E, 128, 2 * 128], U8)  # fp8 diag(ew/c) x2 slices
    din("scl", [128, L * HPC])                     # exp scale c per (l,h)
    din("ident", [128, 128])
    if not zeros["in_b"]:
        din("in_b_r", [128, 4])
    if not zeros["qb"]:
        din("qb_r", [L, 128, 2])                   # pre-scaled by 1/(sqrt(HD)*c_h)
    if not zeros["vb"]:
        din("vb_b", [L, 128, DPC])
    if not zeros["ob"]:
        din("ob_r", [L, 128, 4])
        din("ob_b2", [128, D])
    if not zeros["ln_g"]:
        din("ln_g_b", [128, D])
    if not zeros["ln_b"]:
        din("ln_b_b", [128, D])
    out_d = nc.dram_tensor("out", [N, D], F32, kind="ExternalOutput")

    with tile.TileContext(nc) as tc:
        _emit(nc, tc, dram, out_d, zeros)
    nc.compile()
    return nc


def _emit(nc, tc, dram, out_d, zeros):
    IC = 2  # i-chunks of 512
    CH = N // IC

    with (
        tc.tile_pool(name="res", bufs=1) as res,          # persistent SBUF
        tc.tile_pool(name="wts", bufs=2) as wts,          # per-layer weights (dbl buf)
        tc.tile_pool(name="work", bufs=1) as work,
        tc.tile_pool(name="exp", bufs=3) as expp,
        tc.tile_pool(name="ps_lg", bufs=4, space="PSUM") as ps_lg,
        tc.tile_pool(name="ps_av", bufs=2, space="PSUM") as ps_av,
        tc.tile_pool(name="ps_pj", bufs=2, space="PSUM") as ps_pj,
        tc.tile_pool(name="dram", bufs=2, space="DRAM") as dpool,
    ):
        # ---- small one-time loads (before the big edges DMA) ---------------
        ident = res.tile([128, 128], F32, tag="ident", name="ident")
        nc.sync.dma_start(out=ident[:], in_=dram["ident"][:])
        inw_sb = res.tile([128, D], BF16, tag="inw", name="inw")
        nc.sync.dma_start(out=inw_sb[:], in_=dram["in_w"][:].bitcast(BF16))
        xT_sb = res.tile([128, N], BF16, tag="xT", name="xT")
        nc.sync.dma_start(out=xT_sb[:], in_=dram["xT"][:].bitcast(BF16))
        scl_sb = res.tile([128, L * HPC], F32, tag="scl", name="scl")
        nc.sync.dma_start(out=scl_sb[:], in_=dram["scl"][:])
        if not zeros["in_b"]:
            inb_sb = res.tile([128, 4], F32, tag="inb", name="inb")
            nc.sync.dma_start(out=inb_sb[:], in_=dram["in_b_r"][:])
        if not zeros["ln_g"]:
            lng_sb = res.tile([128, D], F32, tag="lng", name="lng")
            nc.sync.dma_start(out=lng_sb[:], in_=dram["ln_g_b"][:])
        if not zeros["ln_b"]:
            lnb_sb = res.tile([128, D], F32, tag="lnb", name="lnb")
            nc.sync.dma_start(out=lnb_sb[:], in_=dram["ln_b_b"][:])
        if not zeros["ob"]:
            obb2_sb = res.tile([128, D], F32, tag="obb2", name="obb2")
            nc.sync.dma_start(out=obb2_sb[:], in_=dram["ob_b2"][:])

        # ---- per-layer weight loader (wts pool, bufs=2 for prefetch) --------
        def load_weights(l):
            w = {"qw": [], "kw": [], "vw": [], "ow": [], "ew": []}
            for kt in range(4):
                qt = wts.tile([128, DPC], BF16, tag=f"qw{kt}", name=f"qw{kt}")
                nc.sync.dma_start(out=qt[:], in_=dram["qw"][l, kt * 128 : (kt + 1) * 128].bitcast(BF16))
                w["qw"].append(qt)
                ktile = wts.tile([128, DPC], BF16, tag=f"kw{kt}", name=f"kw{kt}")
                nc.sync.dma_start(out=ktile[:], in_=dram["kw"][l, kt * 128 : (kt + 1) * 128].bitcast(BF16))
                w["kw"].append(ktile)
                vt = wts.tile([128, DPC], BF16, tag=f"vw{kt}", name=f"vw{kt}")
                nc.sync.dma_start(out=vt[:], in_=dram["vw"][l, kt * 128 : (kt + 1) * 128].bitcast(BF16))
                w["vw"].append(vt)
                ot = wts.tile([128, D], BF16, tag=f"ow{kt}", name=f"ow{kt}")
                nc.sync.dma_start(out=ot[:], in_=dram["ow"][l, kt * 128 : (kt + 1) * 128].bitcast(BF16))
                w["ow"].append(ot)
            for h in range(HPC):
                row = []
                for e in range(E):
                    t = wts.tile([128, 2 * 128], F8, tag=f"ew{h}_{e}", name=f"ew{h}_{e}")
                    nc.sync.dma_start(
                        out=t[:], in_=dram["ew8_diag"][l, h, e].bitcast(F8)
                    )
                    row.append(t)
                w["ew"].append(row)
            if not zeros["qb"]:
                w["qb"] = wts.tile([128, 2], F32, tag="qb", name="qb")
                nc.sync.dma_start(out=w["qb"][:], in_=dram["qb_r"][l])
            if not zeros["vb"]:
                w["vb"] = wts.tile([128, DPC], F32, tag="vb", name="vb")
                nc.sync.dma_start(out=w["vb"][:], in_=dram["vb_b"][l])
            if not zeros["ob"]:
                w["ob"] = wts.tile([128, 4], F32, tag="ob", name="ob")
                nc.sync.dma_start(out=w["ob"][:], in_=dram["ob_r"][l])
            return w

        n_layers = int(os.environ.get("K_LAYERS", L))
        wcur = load_weights(0)

        # ---- edges: fp8 hi|lo tiles, jt-major so attention can stream ------
        edges_sb = [[None] * E for _ in range(8)]
        for jt in range(8):
            for e in range(E):
                t = res.tile([128, 2 * N], F8, tag=f"edg{jt}_{e}", name=f"edg{jt}_{e}")
                nc.sync.dma_start(out=t[:], in_=dram["edges_f8"][jt, e].bitcast(F8))
                edges_sb[jt][e] = t

        # ---- input projection: hT[dt] = (x @ in_w).T ------------------------
        hT = []
        for dt in range(4):
            t = work.tile([128, N], F32, tag=f"hT{dt}", name=f"hT{dt}")
            for ic in range(IC):
                ps = ps_pj.tile([128, CH], F32, tag="pj", name="pj")
                nc.tensor.matmul(
                    ps[:],
                    inw_sb[:, dt * 128 : (dt + 1) * 128],
                    xT_sb[:, ic * CH : (ic + 1) * CH],
                    start=True,
                    stop=True,
                )
                if zeros["in_b"]:
                    nc.vector.tensor_copy(t[:, ic * CH : (ic + 1) * CH], ps[:])
                else:
                    nc.vector.tensor_scalar_add(
                        t[:, ic * CH : (ic + 1) * CH], ps[:], inb_sb[:, dt : dt + 1]
                    )
            hT.append(t)
        hT_bf = []
        for dt in range(4):
            tb = work.tile([128, N], BF16, tag=f"hTb{dt}", name=f"hTb{dt}")
            nc.vector.tensor_copy(tb[:], hT[dt][:])
            hT_bf.append(tb)

        def _dump_hT_and_exit():
            for it in range(8):
                tt = work.tile([128, D], F32, tag="dump", name="dump", bufs=2)
                for dt2 in range(4):
                    nc.vector.tensor_copy(
                        tt[:, dt2 * 128 : (dt2 + 1) * 128],
                        hT[dt2][:, it * 128 : (it + 1) * 128],
                    )
                nc.sync.dma_start(out=out_d[it * 128 : (it + 1) * 128, :], in_=tt[:])

        if os.environ.get("K_STAGE") == "inproj":
            _dump_hT_and_exit()
            return

        nat = None  # natural-layout h tiles (built at layer L-1)

        # ---- layers ---------------------------------------------------------
        for l in range(n_layers):
            w = wcur

            # ---- transpose hT -> natural h tiles at the last layer ----------
            if l == n_layers - 1:
                nat = []
                for it in range(8):
                    t = work.tile([128, D], F32, tag=f"nat{it}", name=f"nat{it}")
                    nat.append(t)
                for dt in range(4):
                    for it in range(8):
                        ps = ps_pj.tile([128, 128], F32, tag="pj", name="pj")
                        nc.tensor.transpose(
                            ps[:], hT[dt][:, it * 128 : (it + 1) * 128], ident[:]
                        )
                        nc.vector.tensor_copy(
                            nat[it][:, dt * 128 : (dt + 1) * 128], ps[:]
                        )

            # ---- QKV projections -------------------------------------------
            qT, kT = [], []
            for t_i in range(2):
                qt = work.tile([128, N], BF16, tag=f"qT{t_i}", name=f"qT{t_i}")
                ktt = work.tile([128, N], BF16, tag=f"kT{t_i}", name=f"kT{t_i}")
                for ic in range(IC):
                    ps = ps_pj.tile([128, CH], F32, tag="pj", name="pj")
                    for kt in range(4):
                        nc.tensor.matmul(
                            ps[:],
                            w["qw"][kt][:, t_i * 128 : (t_i + 1) * 128],
                            hT_bf[kt][:, ic * CH : (ic + 1) * CH],
                            start=(kt == 0),
                            stop=(kt == 3),
                        )
                    if zeros["qb"]:
                        nc.vector.tensor_copy(qt[:, ic * CH : (ic + 1) * CH], ps[:])
                    else:
                        nc.vector.tensor_scalar_add(
                            qt[:, ic * CH : (ic + 1) * CH], ps[:],
                            w["qb"][:, t_i : t_i + 1],
                        )
                    ps2 = ps_pj.tile([128, CH], F32, tag="pj", name="pj")
                    for kt in range(4):
                        nc.tensor.matmul(
                            ps2[:],
                            w["kw"][kt][:, t_i * 128 : (t_i + 1) * 128],
                            hT_bf[kt][:, ic * CH : (ic + 1) * CH],
                            start=(kt == 0),
                            stop=(kt == 3),
                        )
                    nc.vector.tensor_copy(ktt[:, ic * CH : (ic + 1) * CH], ps2[:])
                qT.append(qt)
                kT.append(ktt)

            v_aug = []
            for jt in range(8):
                vt = work.tile([128, HPC * (HD + 1)], BF16, tag=f"vau{jt}", name=f"vau{jt}")
                v3 = vt[:].rearrange("p (h w) -> p h w", w=HD + 1)
                ps = ps_pj.tile([128, DPC], F32, tag="pj", name="pj")
                for kt in range(4):
                    nc.tensor.matmul(
                        ps[:],
                        hT_bf[kt][:, jt * 128 : (jt + 1) * 128],
                        w["vw"][kt][:],
                        start=(kt == 0),
                        stop=(kt == 3),
                    )
                psv = ps[:].rearrange("p (h w) -> p h w", w=HD)
                if zeros["vb"]:
                    nc.vector.tensor_copy(v3[:, :, 0:HD], psv)
                else:
                    nc.vector.tensor_tensor(
                        v3[:, :, 0:HD], psv,
                        w["vb"][:].rearrange("p (h w) -> p h w", w=HD),
                        op=ALU.add,
                    )
                nc.vector.memset(v3[:, :, HD : HD + 1], 1.0)
                v_aug.append(vt)

            if os.environ.get("K_STAGE") == "qkv":
                _dump_hT_and_exit()
                return

            # prefetch next layer's weights during attention
            if l + 1 < n_layers:
                wnext = load_weights(l + 1)

            # ---- attention (+ pipelined softmax normalization) --------------
            # Two half-collectives: t_i=0 (after heads 0,1) gathers D-blocks
            # {kt0, kt2}; t_i=1 (after heads 2,3) gathers {kt1, kt3}.  The
            # first CC and half of the out-projection overlap heads 2,3.
            cc_in = [
                dpool.tile([1, 128, N], BF16, tag=f"ccin{t}", name=f"ccin{t}")
                for t in range(2)
            ]
            cc_out = [
                dpool.tile([2, 128, N], BF16, tag=f"ccout{t}", name=f"ccout{t}")
                for t in range(2)
            ]
            aof = [None] * 4

            def run_cc(t_i):
                if os.environ.get("K_SKIP_CC"):
                    nc.sync.dma_start(out=cc_out[t_i][0:1], in_=cc_in[t_i][:])
                    nc.sync.dma_start(out=cc_out[t_i][1:2], in_=cc_in[t_i][:])
                else:
                    nc.gpsimd.collective_compute(
                        "AllGather",
                        ALU.bypass,
                        replica_groups=REPLICA_GROUPS,
                        ins=[cc_in[t_i][:].opt()],
                        outs=[cc_out[t_i][:].opt()],
                    )
                for pair in range(2):
                    kt = 2 * pair + t_i
                    t = work.tile([128, N], BF16, tag=f"aof{kt}", name=f"aof{kt}")
                    nc.sync.dma_start(out=t[:], in_=cc_out[t_i][pair])
                    aof[kt] = t

            # partial out-projection over the D-blocks gathered so far;
            # partials accumulate in SBUF (PSUM banks are full of attn state)
            oprt = [None] * 8

            def oproj_part(kts, first):
                if l < n_layers - 1:
                    for dt in range(4):
                        for ic in range(IC):
                            ps = ps_pj.tile([128, CH], F32, tag="pj", name="pj")
                            for i, kt in enumerate(kts):
                                nc.tensor.matmul(
                                    ps[:],
                                    w["ow"][kt][:, dt * 128 : (dt + 1) * 128],
                                    aof[kt][:, ic * CH : (ic + 1) * CH],
                                    start=(i == 0),
                                    stop=(i == len(kts) - 1),
                                )
                            idx = dt * IC + ic
                            if first:
                                t = work.tile(
                                    [128, CH], F32, tag=f"opr{idx}", name=f"opr{idx}"
                                )
                                oprt[idx] = t
                                nc.vector.tensor_copy(t[:], ps[:])
                            else:
                                sl = hT[dt][:, ic * CH : (ic + 1) * CH]
                                if zeros["ob"]:
                                    nc.vector.tensor_tensor(
                                        sl, ps[:], sl, op=ALU.add
                                    )
                                else:
                                    nc.vector.scalar_tensor_tensor(
                                        sl, ps[:], w["ob"][:, dt : dt + 1], sl,
                                        op0=ALU.add, op1=ALU.add,
                                    )
                                nc.vector.tensor_tensor(
                                    sl, oprt[idx][:], sl, op=ALU.add
                                )
                                nc.vector.tensor_copy(
                                    hT_bf[dt][:, ic * CH : (ic + 1) * CH], sl
                                )
                else:
                    # natural layout: h3[it] = nat[it] + aofT @ ow (+ ob)
                    for it in range(8):
                        ps = ps_pj.tile([128, D], F32, tag="pj", name="pj")
                        for i, kt in enumerate(kts):
                            nc.tensor.matmul(
                                ps[:],
                                aof[kt][:, it * 128 : (it + 1) * 128],
                                w["ow"][kt][:],
                                start=(i == 0),
                                stop=(i == len(kts) - 1),
                            )
                        if first:
                            t = work.tile([128, D], F32, tag=f"opr{it}", name=f"opr{it}")
                            oprt[it] = t
                            nc.vector.tensor_copy(t[:], ps[:])
                        else:
                            nc.vector.tensor_tensor(
                                nat[it][:], ps[:], nat[it][:], op=ALU.add
                            )
                            nc.vector.tensor_tensor(
                                nat[it][:], oprt[it][:], nat[it][:], op=ALU.add
                            )
                            if not zeros["ob"]:
                                nc.vector.tensor_tensor(
                                    nat[it][:], nat[it][:], obb2_sb[:], op=ALU.add
                                )

            for h in range(HPC):
                t_i, r0 = h // 2, (h % 2) * 64
                for ic in range(IC):
                    avp = ps_av.tile([HD + 1, CH], F32, tag="av", name="av")
                    # group 4 jt-tiles: [qk x4 bf16][bias x16 fp8 + exp][av x4]
                    # to amortize the PE fp8<->bf16 transition penalty
                    for jg in range(2):
                        jts = range(jg * 4, jg * 4 + 4)
                        lgs, ets = {}, {}
                        for jt in jts:
                            lg = ps_lg.tile([128, CH], F32, tag="lg", name="lg")
                            lgs[jt] = lg
                            nc.tensor.matmul(
                                lg[:],
                                kT[t_i][r0 : r0 + HD, jt * 128 : (jt + 1) * 128],
                                qT[t_i][r0 : r0 + HD, ic * CH : (ic + 1) * CH],
                                start=True,
                                stop=False,
                            )
                        for jt in jts:
                            for e in range(E):
                                em = edges_sb[jt][e][:].rearrange(
                                    "p (two n) -> p two n", two=2
                                )
                                nc.tensor.matmul(
                                    lgs[jt][:],
                                    w["ew"][h][e][:].rearrange(
                                        "p (two f) -> p two f", two=2
                                    ),
                                    em[:, :, ic * CH : (ic + 1) * CH],
                                    start=False,
                                    stop=(e == E - 1),
                                    perf_mode=DR,
                                )
                            et = expp.tile([128, CH], BF16, tag="exp", name="exp")
                            ets[jt] = et
                            nc.scalar.activation(
                                et[:], lgs[jt][:], ACTF.Exp,
                                scale=scl_sb[:, l * HPC + h : l * HPC + h + 1],
                            )
                        for jt in jts:
                            nc.tensor.matmul(
                                avp[:],
                                v_aug[jt][:, h * (HD + 1) : (h + 1) * (HD + 1)],
                                ets[jt][:],
                                start=(jt == 0),
                                stop=(jt == 7),
                            )
                    # pipelined normalize: 1/sums -> broadcast -> fused mul
                    rec = work.tile([1, CH], F32, tag="rec", name="rec", bufs=2)
                    nc.vector.reciprocal(rec[:], avp[HD : HD + 1, :])
                    bc = work.tile([HD, CH], F32, tag="bc", name="bc", bufs=2)
                    nc.gpsimd.partition_broadcast(bc[:], rec[:])
                    aos = work.tile([HD, CH], BF16, tag="aos", name="aos", bufs=3)
                    nc.vector.tensor_tensor(aos[:], avp[0:HD, :], bc[:], op=ALU.mult)
                    nc.sync.dma_start(
                        out=cc_in[t_i][0, r0 : r0 + HD, ic * CH : (ic + 1) * CH],
                        in_=aos[:],
                    )
                # after heads {0,1}: fire CC for t_i=0 and overlap with h2,h3
                if h == 1:
                    run_cc(0)
                    oproj_part([0, 2], first=True)
            run_cc(1)
            oproj_part([1, 3], first=False)
            if l + 1 < n_layers:
                wcur = wnext

        if os.environ.get("K_STAGE") == "oproj":
            _dump_hT_and_exit()
            return

        # ---- final layernorm + store ---------------------------------------
        eps_sb = res.tile([128, 1], F32, tag="eps", name="eps")
        nc.vector.memset(eps_sb[:], LN_EPS)
        for it in range(8):
            s = work.tile([128, 1], F32, tag="lns", name="lns")
            nc.vector.reduce_sum(s[:], nat[it][:], axis=AX.X)
            mean = work.tile([128, 1], F32, tag="lnm", name="lnm")
            nc.scalar.mul(mean[:], s[:], 1.0 / D)
            cent = nat[it]
            nc.vector.tensor_scalar_sub(cent[:], nat[it][:], mean[:])
            ssq = work.tile([128, 1], F32, tag="lnq", name="lnq")
            ot_scratch = work.tile([128, D], F32, tag="lno", name="lno")
            nc.scalar.activation(ot_scratch[:], cent[:], ACTF.Square)
            nc.vector.reduce_sum(ssq[:], ot_scratch[:], axis=AX.X)
            sd = work.tile([128, 1], F32, tag="lnd", name="lnd")
            nc.scalar.activation(
                sd[:], ssq[:], ACTF.Sqrt, scale=1.0 / D, bias=eps_sb[:]
            )
            rstd = work.tile([128, 1], F32, tag="lnr", name="lnr")
            nc.vector.reciprocal(rstd[:], sd[:])
            ot = ot_scratch
            if zeros["ln_g"]:
                nc.vector.tensor_scalar_mul(ot[:], cent[:], rstd[:])
            else:
                nc.vector.scalar_tensor_tensor(
                    ot[:], cent[:], rstd[:], lng_sb[:], op0=ALU.mult, op1=ALU.mult
                )
            if not zeros["ln_b"]:
                nc.vector.tensor_tensor(ot[:], ot[:], lnb_sb[:], op=ALU.add)
            nc.sync.dma_start(out=out_d[it * 128 : (it + 1) * 128, :], in_=ot[:])


_cache = {}


def _get_graph(zeros_key, zeros):
    if zeros_key not in _cache:
        _cache[zeros_key] = _build(zeros)
    return _cache[zeros_key]


def _prep(**inputs):
    x = np.asarray(inputs["x"], dtype=np.float32)
    edges = np.asarray(inputs["edges"], dtype=np.float32)
    in_w = np.asarray(inputs["in_w"], dtype=np.float32)
    in_b = np.asarray(inputs["in_b"], dtype=np.float32)
    qw = np.asarray(inputs["qw"], dtype=np.float32)
    qb = np.asarray(inputs["qb"], dtype=np.float32)
    kw = np.asarray(inputs["kw"], dtype=np.float32)
    vw = np.asarray(inputs["vw"], dtype=np.float32)
    vb = np.asarray(inputs["vb"], dtype=np.float32)
    ow = np.asarray(inputs["ow"], dtype=np.float32)
    ob = np.asarray(inputs["ob"], dtype=np.float32)
    ew = np.asarray(inputs["ew"], dtype=np.float32)
    ln_g = np.asarray(inputs["ln_g"], dtype=np.float32)
    ln_b = np.asarray(inputs["ln_b"], dtype=np.float32)

    zeros = {
        "in_b": bool(np.all(in_b == 0)),
        "qb": bool(np.all(qb == 0)),
        "vb": bool(np.all(vb == 0)),
        "ob": bool(np.all(ob == 0)),
        "ln_g": bool(np.all(ln_g == 1.0)),
        "ln_b": bool(np.all(ln_b == 0)),
    }
    zeros_key = tuple(sorted(zeros.items()))

    s = np.float32(1.0 / np.sqrt(HD))
    ident = np.eye(128, dtype=np.float32)
    eye = np.eye(128, dtype=np.float32)

    # per-(l, global head) optimal fp8 scale for ew
    c_lh = np.empty((L, H), dtype=np.float32)
    for l in range(L):
        for hh in range(H):
            c_lh[l, hh] = _opt_scale(ew[l, :, hh])

    # per-head scale folded into qw/qb: divide head hh's 64 dims by c
    inv_c_dims = np.repeat(c_lh, HD, axis=1)       # (L, D) per-output-dim c
    qw_s = qw * s / inv_c_dims[:, None, :]
    qb_s = qb * s / inv_c_dims

    in_maps = []
    for c in range(NCORES):
        b, g = c // 2, c % 2
        hs = slice(g * DPC, (g + 1) * DPC)
        # edges -> fp8 hi|lo per (jt, e): [8, E, 128, 2*N]
        tmp = edges[b].transpose(2, 1, 0)            # [e, j, i]
        hi8 = _f8(tmp)
        hi = hi8.astype(np.float32)
        lo8 = _f8(tmp - hi)
        ef8 = np.empty((8, E, 128, 2 * N), dtype=np.uint8)
        for jt in range(8):
            ef8[jt, :, :, 0:N] = hi8.view(np.uint8)[:, jt * 128 : (jt + 1) * 128, :]
            ef8[jt, :, :, N:] = lo8.view(np.uint8)[:, jt * 128 : (jt + 1) * 128, :]
        # fp8 diag(ew/c), duplicated on both DoubleRow slices
        ewd = np.empty((L, HPC, E, 128, 2 * 128), dtype=np.uint8)
        for l in range(L):
            for hl in range(HPC):
                hh = g * HPC + hl
                for e in range(E):
                    dbits = _f8_bits(eye * (ew[l, e, hh] / c_lh[l, hh]))
                    ewd[l, hl, e, :, 0:128] = dbits
                    ewd[l, hl, e, :, 128:] = dbits
        scl = np.broadcast_to(
            c_lh[:, g * HPC : (g + 1) * HPC].reshape(1, L * HPC), (128, L * HPC)
        )
        m = {
            "xT": _bf16_bits(x[b].T),
            "edges_f8": ef8,
            "in_w": _bf16_bits(in_w),
            "qw": _bf16_bits(qw_s[:, :, hs]),
            "kw": _bf16_bits(kw[:, :, hs]),
            "vw": _bf16_bits(vw[:, :, hs]),
            "ow": _bf16_bits(ow),
            "ew8_diag": ewd,
            "scl": np.ascontiguousarray(scl, dtype=np.float32),
            "ident": ident,
        }
        if not zeros["in_b"]:
            m["in_b_r"] = np.ascontiguousarray(in_b.reshape(4, 128).T)
        if not zeros["qb"]:
            m["qb_r"] = np.ascontiguousarray(
                qb_s[:, hs].reshape(L, 2, 128).transpose(0, 2, 1)
            )
        if not zeros["vb"]:
            m["vb_b"] = np.ascontiguousarray(
                np.broadcast_to(vb[:, None, hs], (L, 128, DPC))
            )
        if not zeros["ob"]:
            m["ob_r"] = np.ascontiguousarray(ob.reshape(L, 4, 128).transpose(0, 2, 1))
            m["ob_b2"] = np.ascontiguousarray(np.broadcast_to(ob[L - 1], (128, D)))
        if not zeros["ln_g"]:
            m["ln_g_b"] = np.ascontiguousarray(np.broadcast_to(ln_g, (128, D)))
        if not zeros["ln_b"]:
            m["ln_b_b"] = np.ascontiguousarray(np.broadcast_to(ln_b, (128, D)))
        in_maps.append(m)
    return zeros_key, zeros, in_maps


LAST_RESULT = None


def kernel(**inputs) -> np.ndarray:
    global LAST_RESULT
    zeros_key, zeros, in_maps = _prep(**inputs)
    nc = _get_graph(zeros_key, zeros)
    trace = bool(os.environ.get("K_TRACE"))
    res = run_bass_kernel_spmd(
        nc, in_maps, core_ids=list(range(NCORES)), trace=trace
    )
    LAST_RESULT = res
    out = np.empty((B, N, D), dtype=np.float32)
    for b in range(B):
        out[b] = res.results[2 * b]["out"]
    return out


# revision 8
# speedup vs baseline: 1.0920x; 1.0920x over previous
"""Distributed Trainium2 Bass kernel for the AIRS-GAT problem.

Sharding (8 cores): core c -> (batch b = c//2, head-group g = c%2).
Each core handles one batch and 4 of the 8 heads (tensor-parallel heads),
with an AllGather of the per-head attention outputs inside each core pair
before a (redundant) full output projection, per the sharding hint.

Key design points:
  * h is kept TRANSPOSED on-chip: hT[d, i] as 4 tiles of [128, 1024].
    QKV projections then take natural-layout weights as stationary operands.
  * Attention is computed transposed: logitsT[j, i] so softmax reduction
    over keys j is handled by PE matmuls (sum) and the normalization is
    deferred past the attn@V matmul (ones-row augmented V gives the sums).
  * Softmax skips max-subtraction (validated: max|logit| ~ 11.4).
  * Edge bias (B,N,N,E)@(E,H) runs on the TensorEngine in fp8e4m3
    DoubleRow mode: edges are split host-side into hi+lo fp8 components
    (exact to ~1e-3) that ride the two DoubleRow slices of one matmul, so
    each e costs 256 PE cycles instead of 512.  ew is quantized to fp8
    with a per-(layer,head) optimal scale c; the compensation c is folded
    into the exp activation's scale operand and 1/c into qw host-side.
  * Per-(head,i-chunk) softmax normalization is pipelined: reciprocal of
    the augmented-row sums -> gpsimd partition broadcast -> fused
    multiply+cast straight out of PSUM, then DMA into the collective
    staging buffer, all overlapped with the next tile's attention math.
  * Matmuls run in bf16 (full-speed); kb and eb are dropped (softmax
    shift invariance).
"""

import os
import sys

import numpy as np

for _p in ("/opt/trn_rl_repo",):
    if _p not in sys.path:
        sys.path.insert(0, _p)

import concourse.bass as bass
import concourse.bacc as bacc
import concourse.mybir as mybir
import concourse.tile as tile
from concourse.bass_utils import run_bass_kernel_spmd

B, N, IN_DIM, D, H, E, L = 4, 1024, 128, 512, 8, 4, 3
HD = D // H          # 64
HPC = H // 2         # heads per core = 4
DPC = HPC * HD       # head dims per core = 256
LN_EPS = 1e-5
NCORES = 8
REPLICA_GROUPS = [[0, 1], [2, 3], [4, 5], [6, 7]]

F32 = mybir.dt.float32
F32R = mybir.dt.float32r
BF16 = mybir.dt.bfloat16
F8 = mybir.dt.float8e4
U16 = mybir.dt.uint16
U8 = mybir.dt.uint8
AX = mybir.AxisListType
ALU = mybir.AluOpType
ACTF = mybir.ActivationFunctionType
DR = mybir.MatmulPerfMode.DoubleRow


def _bf16_bits(x: np.ndarray) -> np.ndarray:
    """fp32 -> bf16 bits (round to nearest even), as uint16."""
    u = np.ascontiguousarray(x, dtype=np.float32).view(np.uint32)
    r = ((u >> 16) & 1) + np.uint32(0x7FFF)
    return ((u + r) >> 16).astype(np.uint16)


def _f8(x: np.ndarray) -> np.ndarray:
    import ml_dtypes

    return np.ascontiguousarray(x, dtype=np.float32).astype(ml_dtypes.float8_e4m3fn)


def _f8_bits(x: np.ndarray) -> np.ndarray:
    return _f8(x).view(np.uint8)


def _opt_scale(ews: np.ndarray) -> float:
    """Scale c minimizing sum_e (fp8(ew_e/c)*c - ew_e)^2."""
    cs = np.geomspace(0.25, 4.0, 400)
    best, bc = None, 1.0
    for c in cs:
        err = float(((_f8(ews / c).astype(np.float32) * c - ews) ** 2).sum())
        if best is None or err < best:
            best, bc = err, float(c)
    return bc


def _build(zeros: dict) -> bass.Bass:
    """Build the SPMD kernel graph (identical program for all 8 cores)."""
    nc = bacc.Bacc(
        "TRN2", target_bir_lowering=False, debug=False, num_devices=NCORES
    )

    dram = {}

    def din(name, shape, dtype=F32):
        dram[name] = nc.dram_tensor(name, list(shape), dtype, kind="ExternalInput")
        return dram[name]

    din("xT", [IN_DIM, N], U16)
    din("edges_f8", [8, E, 128, 2 * N], U8)        # [jt, e, j_in_tile, (hi|lo) i]
    din("in_w", [IN_DIM, D], U16)
    din("qw", [L, D, DPC], U16)                    # pre-scaled by 1/(sqrt(HD)*c_h)
    din("kw", [L, D, DPC], U16)
    din("vw", [L, D, DPC], U16)
    din("ow", [L, D, D], U16)                      # full out-proj weight
    din("ew8_diag", [L, HPC, E, 128, 2 * 128], U8)  # fp8 diag(ew/c) x2 slices
    din("scl", [128, L * HPC])                     # exp scale c per (l,h)
    din("ident", [128, 128])
    if not zeros["in_b"]:
        din("in_b_r", [128, 4])
    if not zeros["qb"]:
        din("qb_r", [L, 128, 2])                   # pre-scaled by 1/(sqrt(HD)*c_h)
    if not zeros["vb"]:
        din("vb_b", [L, 128, DPC])
    if not zeros["ob"]:
        din("ob_r", [L, 128, 4])
        din("ob_b2", [128, D])
    if not zeros["ln_g"]:
        din("ln_g_b", [128, D])
    if not zeros["ln_b"]:
        din("ln_b_b", [128, D])
    out_d = nc.dram_tensor("out", [N, D], F32, kind="ExternalOutput")

    with tile.TileContext(nc) as tc:
        _emit(nc, tc, dram, out_d, zeros)
    nc.compile()
    return nc


def _emit(nc, tc, dram, out_d, zeros):
    IC = 2  # i-chunks of 512
    CH = N // IC

    with (
        tc.tile_pool(name="res", bufs=1) as res,          # persistent SBUF
        tc.tile_pool(name="wts", bufs=2) as wts,          # per-layer weights (dbl buf)
        tc.tile_pool(name="work", bufs=1) as work,
        tc.tile_pool(name="exp", bufs=3) as expp,
        tc.tile_pool(name="ps_lg", bufs=4, space="PSUM") as ps_lg,
        tc.tile_pool(name="ps_av", bufs=2, space="PSUM") as ps_av,
        tc.tile_pool(name="ps_pj", bufs=2, space="PSUM") as ps_pj,
        tc.tile_pool(name="dram", bufs=2, space="DRAM") as dpool,
    ):
        # ---- small one-time loads (before the big edges DMA) ---------------
        ident = res.tile([128, 128], F32, tag="ident", name="ident")
        nc.sync.dma_start(out=ident[:], in_=dram["ident"][:])
        inw_sb = res.tile([128, D], BF16, tag="inw", name="inw")
        nc.sync.dma_start(out=inw_sb[:], in_=dram["in_w"][:].bitcast(BF16))
        xT_sb = res.tile([128, N], BF16, tag="xT", name="xT")
        nc.sync.dma_start(out=xT_sb[:], in_=dram["xT"][:].bitcast(BF16))
        scl_sb = res.tile([128, L * HPC], F32, tag="scl", name="scl")
        nc.sync.dma_start(out=scl_sb[:], in_=dram["scl"][:])
        if not zeros["in_b"]:
            inb_sb = res.tile([128, 4], F32, tag="inb", name="inb")
            nc.sync.dma_start(out=inb_sb[:], in_=dram["in_b_r"][:])
        if not zeros["ln_g"]:
            lng_sb = res.tile([128, D], F32, tag="lng", name="lng")
            nc.sync.dma_start(out=lng_sb[:], in_=dram["ln_g_b"][:])
        if not zeros["ln_b"]:
            lnb_sb = res.tile([128, D], F32, tag="lnb", name="lnb")
            nc.sync.dma_start(out=lnb_sb[:], in_=dram["ln_b_b"][:])
        if not zeros["ob"]:
            obb2_sb = res.tile([128, D], F32, tag="obb2", name="obb2")
            nc.sync.dma_start(out=obb2_sb[:], in_=dram["ob_b2"][:])

        # ---- per-layer weight loader (wts pool, bufs=2 for prefetch) --------
        def load_weights(l):
            w = {"qw": [], "kw": [], "vw": [], "ow": [], "ew": []}
            for kt in range(4):
                qt = wts.tile([128, DPC], BF16, tag=f"qw{kt}", name=f"qw{kt}")
                nc.sync.dma_start(out=qt[:], in_=dram["qw"][l, kt * 128 : (kt + 1) * 128].bitcast(BF16))
                w["qw"].append(qt)
                ktile = wts.tile([128, DPC], BF16, tag=f"kw{kt}", name=f"kw{kt}")
                nc.sync.dma_start(out=ktile[:], in_=dram["kw"][l, kt * 128 : (kt + 1) * 128].bitcast(BF16))
                w["kw"].append(ktile)
                vt = wts.tile([128, DPC], BF16, tag=f"vw{kt}", name=f"vw{kt}")
                nc.sync.dma_start(out=vt[:], in_=dram["vw"][l, kt * 128 : (kt + 1) * 128].bitcast(BF16))
                w["vw"].append(vt)
                ot = wts.tile([128, D], BF16, tag=f"ow{kt}", name=f"ow{kt}")
                nc.sync.dma_start(out=ot[:], in_=dram["ow"][l, kt * 128 : (kt + 1) * 128].bitcast(BF16))
                w["ow"].append(ot)
            for h in range(HPC):
                row = []
                for e in range(E):
                    t = wts.tile([128, 2 * 128], F8, tag=f"ew{h}_{e}", name=f"ew{h}_{e}")
                    nc.sync.dma_start(
                        out=t[:], in_=dram["ew8_diag"][l, h, e].bitcast(F8)
                    )
                    row.append(t)
                w["ew"].append(row)
            if not zeros["qb"]:
                w["qb"] = wts.tile([128, 2], F32, tag="qb", name="qb")
                nc.sync.dma_start(out=w["qb"][:], in_=dram["qb_r"][l])
            if not zeros["vb"]:
                w["vb"] = wts.tile([128, DPC], F32, tag="vb", name="vb")
                nc.sync.dma_start(out=w["vb"][:], in_=dram["vb_b"][l])
            if not zeros["ob"]:
                w["ob"] = wts.tile([128, 4], F32, tag="ob", name="ob")
                nc.sync.dma_start(out=w["ob"][:], in_=dram["ob_r"][l])
            return w

        eps_sb = res.tile([128, 1], F32, tag="eps", name="eps")
        nc.vector.memset(eps_sb[:], LN_EPS)

        n_layers = int(os.environ.get("K_LAYERS", L))
        wcur = load_weights(0)

        # ---- edges: fp8 hi|lo tiles, jt-major so attention can stream ------
        edges_sb = [[None] * E for _ in range(8)]
        for jt in range(8):
            for e in range(E):
                t = res.tile([128, 2 * N], F8, tag=f"edg{jt}_{e}", name=f"edg{jt}_{e}")
                nc.sync.dma_start(out=t[:], in_=dram["edges_f8"][jt, e].bitcast(F8))
                edges_sb[jt][e] = t

        # ---- input projection: hT[dt] = (x @ in_w).T ------------------------
        hT = []
        for dt in range(4):
            t = work.tile([128, N], F32, tag=f"hT{dt}", name=f"hT{dt}")
            for ic in range(IC):
                ps = ps_pj.tile([128, CH], F32, tag="pj", name="pj")
                nc.tensor.matmul(
                    ps[:],
                    inw_sb[:, dt * 128 : (dt + 1) * 128],
                    xT_sb[:, ic * CH : (ic + 1) * CH],
                    start=True,
                    stop=True,
                )
                if zeros["in_b"]:
                    nc.vector.tensor_copy(t[:, ic * CH : (ic + 1) * CH], ps[:])
                else:
                    nc.vector.tensor_scalar_add(
                        t[:, ic * CH : (ic + 1) * CH], ps[:], inb_sb[:, dt : dt + 1]
                    )
            hT.append(t)
        hT_bf = []
        for dt in range(4):
            tb = work.tile([128, N], BF16, tag=f"hTb{dt}", name=f"hTb{dt}")
            nc.vector.tensor_copy(tb[:], hT[dt][:])
            hT_bf.append(tb)

        def _dump_hT_and_exit():
            for it in range(8):
                tt = work.tile([128, D], F32, tag="dump", name="dump", bufs=2)
                for dt2 in range(4):
                    nc.vector.tensor_copy(
                        tt[:, dt2 * 128 : (dt2 + 1) * 128],
                        hT[dt2][:, it * 128 : (it + 1) * 128],
                    )
                nc.sync.dma_start(out=out_d[it * 128 : (it + 1) * 128, :], in_=tt[:])

        if os.environ.get("K_STAGE") == "inproj":
            _dump_hT_and_exit()
            return

        nat = None  # natural-layout h tiles (built at layer L-1)

        # ---- layers ---------------------------------------------------------
        for l in range(n_layers):
            w = wcur

            # ---- transpose hT -> natural h tiles at the last layer ----------
            if l == n_layers - 1:
                nat = []
                for it in range(8):
                    t = work.tile([128, D], F32, tag=f"nat{it}", name=f"nat{it}")
                    nat.append(t)
                for dt in range(4):
                    for it in range(8):
                        ps = ps_pj.tile([128, 128], F32, tag="pj", name="pj")
                        nc.tensor.transpose(
                            ps[:], hT[dt][:, it * 128 : (it + 1) * 128], ident[:]
                        )
                        nc.vector.tensor_copy(
                            nat[it][:, dt * 128 : (dt + 1) * 128], ps[:]
                        )

            # ---- QKV projections -------------------------------------------
            qT, kT = [], []
            for t_i in range(2):
                qt = work.tile([128, N], BF16, tag=f"qT{t_i}", name=f"qT{t_i}")
                ktt = work.tile([128, N], BF16, tag=f"kT{t_i}", name=f"kT{t_i}")
                for ic in range(IC):
                    ps = ps_pj.tile([128, CH], F32, tag="pj", name="pj")
                    for kt in range(4):
                        nc.tensor.matmul(
                            ps[:],
                            w["qw"][kt][:, t_i * 128 : (t_i + 1) * 128],
                            hT_bf[kt][:, ic * CH : (ic + 1) * CH],
                            start=(kt == 0),
                            stop=(kt == 3),
                        )
                    if zeros["qb"]:
                        nc.vector.tensor_copy(qt[:, ic * CH : (ic + 1) * CH], ps[:])
                    else:
                        nc.vector.tensor_scalar_add(
                            qt[:, ic * CH : (ic + 1) * CH], ps[:],
                            w["qb"][:, t_i : t_i + 1],
                        )
                    ps2 = ps_pj.tile([128, CH], F32, tag="pj", name="pj")
                    for kt in range(4):
                        nc.tensor.matmul(
                            ps2[:],
                            w["kw"][kt][:, t_i * 128 : (t_i + 1) * 128],
                            hT_bf[kt][:, ic * CH : (ic + 1) * CH],
                            start=(kt == 0),
                            stop=(kt == 3),
                        )
                    nc.vector.tensor_copy(ktt[:, ic * CH : (ic + 1) * CH], ps2[:])
                qT.append(qt)
                kT.append(ktt)

            v_aug = []
            for jt in range(8):
                vt = work.tile([128, HPC * (HD + 1)], BF16, tag=f"vau{jt}", name=f"vau{jt}")
                v3 = vt[:].rearrange("p (h w) -> p h w", w=HD + 1)
                ps = ps_pj.tile([128, DPC], F32, tag="pj", name="pj")
                for kt in range(4):
                    nc.tensor.matmul(
                        ps[:],
                        hT_bf[kt][:, jt * 128 : (jt + 1) * 128],
                        w["vw"][kt][:],
                        start=(kt == 0),
                        stop=(kt == 3),
                    )
                psv = ps[:].rearrange("p (h w) -> p h w", w=HD)
                if zeros["vb"]:
                    nc.vector.tensor_copy(v3[:, :, 0:HD], psv)
                else:
                    nc.vector.tensor_tensor(
                        v3[:, :, 0:HD], psv,
                        w["vb"][:].rearrange("p (h w) -> p h w", w=HD),
                        op=ALU.add,
                    )
                nc.vector.memset(v3[:, :, HD : HD + 1], 1.0)
                v_aug.append(vt)

            if os.environ.get("K_STAGE") == "qkv":
                _dump_hT_and_exit()
                return

            # prefetch next layer's weights during attention
            if l + 1 < n_layers:
                wnext = load_weights(l + 1)

            # ---- attention (+ pipelined softmax normalization) --------------
            # Two half-collectives: t_i=0 (after heads 0,1) gathers D-blocks
            # {kt0, kt2}; t_i=1 (after heads 2,3) gathers {kt1, kt3}.  The
            # first CC and half of the out-projection overlap heads 2,3.
            cc_in = [
                dpool.tile([1, 128, N], BF16, tag=f"ccin{t}", name=f"ccin{t}")
                for t in range(2)
            ]
            cc_out = [
                dpool.tile([2, 128, N], BF16, tag=f"ccout{t}", name=f"ccout{t}")
                for t in range(2)
            ]
            aof = [None] * 4

            def run_cc(t_i):
                if os.environ.get("K_SKIP_CC"):
                    nc.sync.dma_start(out=cc_out[t_i][0:1], in_=cc_in[t_i][:])
                    nc.sync.dma_start(out=cc_out[t_i][1:2], in_=cc_in[t_i][:])
                else:
                    nc.gpsimd.collective_compute(
                        "AllGather",
                        ALU.bypass,
                        replica_groups=REPLICA_GROUPS,
                        ins=[cc_in[t_i][:].opt()],
                        outs=[cc_out[t_i][:].opt()],
                    )
                for pair in range(2):
                    kt = 2 * pair + t_i
                    t = work.tile([128, N], BF16, tag=f"aof{kt}", name=f"aof{kt}")
                    nc.sync.dma_start(out=t[:], in_=cc_out[t_i][pair])
                    aof[kt] = t

            # split out-projection: part1 (kt 0,2 after the first CC) adds
            # straight into the fp32 residual early; part2 (kt 1,3) finishes
            # it and re-casts hT_bf (cast on the scalar engine to keep the
            # layer-end DVE chain short)
            def oproj_part(kts, first):
                if l < n_layers - 1:
                    for dt in range(4):
                        for ic in range(IC):
                            ps = ps_pj.tile([128, CH], F32, tag="pj", name="pj")
                            for i, kt in enumerate(kts):
                                nc.tensor.matmul(
                                    ps[:],
                                    w["ow"][kt][:, dt * 128 : (dt + 1) * 128],
                                    aof[kt][:, ic * CH : (ic + 1) * CH],
                                    start=(i == 0),
                                    stop=(i == len(kts) - 1),
                                )
                            sl = hT[dt][:, ic * CH : (ic + 1) * CH]
                            if first and not zeros["ob"]:
                                nc.vector.scalar_tensor_tensor(
                                    sl, ps[:], w["ob"][:, dt : dt + 1], sl,
                                    op0=ALU.add, op1=ALU.add,
                                )
                            else:
                                nc.vector.tensor_tensor(sl, ps[:], sl, op=ALU.add)
                            if not first:
                                nc.scalar.copy(
                                    hT_bf[dt][:, ic * CH : (ic + 1) * CH], sl
                                )
                else:
                    # natural layout: h3[it] = nat[it] + aofT @ ow (+ ob)
                    for it in range(8):
                        ps = ps_pj.tile([128, D], F32, tag="pj", name="pj")
                        for i, kt in enumerate(kts):
                            nc.tensor.matmul(
                                ps[:],
                                aof[kt][:, it * 128 : (it + 1) * 128],
                                w["ow"][kt][:],
                                start=(i == 0),
                                stop=(i == len(kts) - 1),
                            )
                        nc.vector.tensor_tensor(
                            nat[it][:], ps[:], nat[it][:], op=ALU.add
                        )
                        if first and not zeros["ob"]:
                            nc.vector.tensor_tensor(
                                nat[it][:], nat[it][:], obb2_sb[:], op=ALU.add
                            )
                        if not first:
                            _layernorm_tile(it)

            def _layernorm_tile(it):
                s = work.tile([128, 1], F32, tag="lns", name="lns", bufs=2)
                nc.vector.reduce_sum(s[:], nat[it][:], axis=AX.X)
                mean = work.tile([128, 1], F32, tag="lnm", name="lnm", bufs=2)
                nc.scalar.mul(mean[:], s[:], 1.0 / D)
                cent = nat[it]
                nc.vector.tensor_scalar_sub(cent[:], nat[it][:], mean[:])
                ssq = work.tile([128, 1], F32, tag="lnq", name="lnq", bufs=2)
                ot_scratch = work.tile([128, D], F32, tag="lno", name="lno", bufs=2)
                nc.scalar.activation(
                    ot_scratch[:], cent[:], ACTF.Square, accum_out=ssq[:]
                )
                sd = work.tile([128, 1], F32, tag="lnd", name="lnd", bufs=2)
                nc.scalar.activation(
                    sd[:], ssq[:], ACTF.Sqrt, scale=1.0 / D, bias=eps_sb[:]
                )
                rstd = work.tile([128, 1], F32, tag="lnr", name="lnr", bufs=2)
                nc.vector.reciprocal(rstd[:], sd[:])
                ot = ot_scratch
                if zeros["ln_g"]:
                    nc.vector.tensor_scalar_mul(ot[:], cent[:], rstd[:])
                else:
                    nc.vector.scalar_tensor_tensor(
                        ot[:], cent[:], rstd[:], lng_sb[:], op0=ALU.mult, op1=ALU.mult
                    )
                if not zeros["ln_b"]:
                    nc.vector.tensor_tensor(ot[:], ot[:], lnb_sb[:], op=ALU.add)
                nc.sync.dma_start(out=out_d[it * 128 : (it + 1) * 128, :], in_=ot[:])

            for h in range(HPC):
                t_i, r0 = h // 2, (h % 2) * 64
                for ic in range(IC):
                    avp = ps_av.tile([HD + 1, CH], F32, tag="av", name="av")
                    # group 4 jt-tiles: [qk x4 bf16][bias x16 fp8 + exp][av x4]
                    # to amortize the PE fp8<->bf16 transition penalty
                    for jg in range(2):
                        jts = range(jg * 4, jg * 4 + 4)
                        lgs, ets = {}, {}
                        for jt in jts:
                            lg = ps_lg.tile([128, CH], F32, tag="lg", name="lg")
                            lgs[jt] = lg
                            nc.tensor.matmul(
                                lg[:],
                                kT[t_i][r0 : r0 + HD, jt * 128 : (jt + 1) * 128],
                                qT[t_i][r0 : r0 + HD, ic * CH : (ic + 1) * CH],
                                start=True,
                                stop=False,
                            )
                        for jt in jts:
                            for e in range(E):
                                em = edges_sb[jt][e][:].rearrange(
                                    "p (two n) -> p two n", two=2
                                )
                                nc.tensor.matmul(
                                    lgs[jt][:],
                                    w["ew"][h][e][:].rearrange(
                                        "p (two f) -> p two f", two=2
                                    ),
                                    em[:, :, ic * CH : (ic + 1) * CH],
                                    start=False,
                                    stop=(e == E - 1),
                                    perf_mode=DR,
                                )
                            et = expp.tile([128, CH], BF16, tag="exp", name="exp")
                            ets[jt] = et
                            nc.scalar.activation(
                                et[:], lgs[jt][:], ACTF.Exp,
                                scale=scl_sb[:, l * HPC + h : l * HPC + h + 1],
                            )
                        for jt in jts:
                            nc.tensor.matmul(
                                avp[:],
                                v_aug[jt][:, h * (HD + 1) : (h + 1) * (HD + 1)],
                                ets[jt][:],
                                start=(jt == 0),
                                stop=(jt == 7),
                            )
                    # pipelined normalize: 1/sums -> broadcast -> fused mul
                    rec = work.tile([1, CH], F32, tag="rec", name="rec", bufs=2)
                    nc.vector.reciprocal(rec[:], avp[HD : HD + 1, :])
                    bc = work.tile([HD, CH], F32, tag="bc", name="bc", bufs=2)
                    nc.gpsimd.partition_broadcast(bc[:], rec[:])
                    aos = work.tile([HD, CH], BF16, tag="aos", name="aos", bufs=3)
                    nc.vector.tensor_tensor(aos[:], avp[0:HD, :], bc[:], op=ALU.mult)
                    nc.sync.dma_start(
                        out=cc_in[t_i][0, r0 : r0 + HD, ic * CH : (ic + 1) * CH],
                        in_=aos[:],
                    )
                # after heads {0,1}: fire CC for t_i=0 (overlaps h2,h3);
                # after h2: run the first half of the out-projection (its CC
                # finished during h2, so the PE never stalls on it)
                if h == 1:
                    run_cc(0)
                if h == 2:
                    oproj_part([0, 2], first=True)
            run_cc(1)
            oproj_part([1, 3], first=False)
            if l + 1 < n_layers:
                wcur = wnext

        if os.environ.get("K_STAGE") == "oproj":
            _dump_hT_and_exit()
            return



_cache = {}


def _get_graph(zeros_key, zeros):
    if zeros_key not in _cache:
        _cache[zeros_key] = _build(zeros)
    return _cache[zeros_key]


def _prep(**inputs):
    x = np.asarray(inputs["x"], dtype=np.float32)
    edges = np.asarray(inputs["edges"], dtype=np.float32)
    in_w = np.asarray(inputs["in_w"], dtype=np.float32)
    in_b = np.asarray(inputs["in_b"], dtype=np.float32)
    qw = np.asarray(inputs["qw"], dtype=np.float32)
    qb = np.asarray(inputs["qb"], dtype=np.float32)
    kw = np.asarray(inputs["kw"], dtype=np.float32)
    vw = np.asarray(inputs["vw"], dtype=np.float32)
    vb = np.asarray(inputs["vb"], dtype=np.float32)
    ow = np.asarray(inputs["ow"], dtype=np.float32)
    ob = np.asarray(inputs["ob"], dtype=np.float32)
    ew = np.asarray(inputs["ew"], dtype=np.float32)
    ln_g = np.asarray(inputs["ln_g"], dtype=np.float32)
    ln_b = np.asarray(inputs["ln_b"], dtype=np.float32)

    zeros = {
        "in_b": bool(np.all(in_b == 0)),
        "qb": bool(np.all(qb == 0)),
        "vb": bool(np.all(vb == 0)),
        "ob": bool(np.all(ob == 0)),
        "ln_g": bool(np.all(ln_g == 1.0)),
        "ln_b": bool(np.all(ln_b == 0)),
    }
    zeros_key = tuple(sorted(zeros.items()))

    s = np.float32(1.0 / np.sqrt(HD))
    ident = np.eye(128, dtype=np.float32)
    eye = np.eye(128, dtype=np.float32)

    # per-(l, global head) optimal fp8 scale for ew
    c_lh = np.empty((L, H), dtype=np.float32)
    for l in range(L):
        for hh in range(H):
            c_lh[l, hh] = _opt_scale(ew[l, :, hh])

    # per-head scale folded into qw/qb: divide head hh's 64 dims by c
    inv_c_dims = np.repeat(c_lh, HD, axis=1)       # (L, D) per-output-dim c
    qw_s = qw * s / inv_c_dims[:, None, :]
    qb_s = qb * s / inv_c_dims

    in_maps = []
    for c in range(NCORES):
        b, g = c // 2, c % 2
        hs = slice(g * DPC, (g + 1) * DPC)
        # edges -> fp8 hi|lo per (jt, e): [8, E, 128, 2*N]
        tmp = edges[b].transpose(2, 1, 0)            # [e, j, i]
        hi8 = _f8(tmp)
        hi = hi8.astype(np.float32)
        lo8 = _f8(tmp - hi)
        ef8 = np.empty((8, E, 128, 2 * N), dtype=np.uint8)
        for jt in range(8):
            ef8[jt, :, :, 0:N] = hi8.view(np.uint8)[:, jt * 128 : (jt + 1) * 128, :]
            ef8[jt, :, :, N:] = lo8.view(np.uint8)[:, jt * 128 : (jt + 1) * 128, :]
        # fp8 diag(ew/c), duplicated on both DoubleRow slices
        ewd = np.empty((L, HPC, E, 128, 2 * 128), dtype=np.uint8)
        for l in range(L):
            for hl in range(HPC):
                hh = g * HPC + hl
                for e in range(E):
                    dbits = _f8_bits(eye * (ew[l, e, hh] / c_lh[l, hh]))
                    ewd[l, hl, e, :, 0:128] = dbits
                    ewd[l, hl, e, :, 128:] = dbits
        scl = np.broadcast_to(
            c_lh[:, g * HPC : (g + 1) * HPC].reshape(1, L * HPC), (128, L * HPC)
        )
        m = {
            "xT": _bf16_bits(x[b].T),
            "edges_f8": ef8,
            "in_w": _bf16_bits(in_w),
            "qw": _bf16_bits(qw_s[:, :, hs]),
            "kw": _bf16_bits(kw[:, :, hs]),
            "vw": _bf16_bits(vw[:, :, hs]),
            "ow": _bf16_bits(ow),
            "ew8_diag": ewd,
            "scl": np.ascontiguousarray(scl, dtype=np.float32),
            "ident": ident,
        }
        if not zeros["in_b"]:
            m["in_b_r"] = np.ascontiguousarray(in_b.reshape(4, 128).T)
        if not zeros["qb"]:
            m["qb_r"] = np.ascontiguousarray(
                qb_s[:, hs].reshape(L, 2, 128).transpose(0, 2, 1)
            )
        if not zeros["vb"]:
            m["vb_b"] = np.ascontiguousarray(
                np.broadcast_to(vb[:, None, hs], (L, 128, DPC))
            )
        if not zeros["ob"]:
            m["ob_r"] = np.ascontiguousarray(ob.reshape(L, 4, 128).transpose(0, 2, 1))
            m["ob_b2"] = np.ascontiguousarray(np.broadcast_to(ob[L - 1], (128, D)))
        if not zeros["ln_g"]:
            m["ln_g_b"] = np.ascontiguousarray(np.broadcast_to(ln_g, (128, D)))
        if not zeros["ln_b"]:
            m["ln_b_b"] = np.ascontiguousarray(np.broadcast_to(ln_b, (128, D)))
        in_maps.append(m)
    return zeros_key, zeros, in_maps


LAST_RESULT = None


def kernel(**inputs) -> np.ndarray:
    global LAST_RESULT
    zeros_key, zeros, in_maps = _prep(**inputs)
    nc = _get_graph(zeros_key, zeros)
    trace = bool(os.environ.get("K_TRACE"))
    res = run_bass_kernel_spmd(
        nc, in_maps, core_ids=list(range(NCORES)), trace=trace
    )
    LAST_RESULT = res
    out = np.empty((B, N, D), dtype=np.float32)
    for b in range(B):
        out[b] = res.results[2 * b]["out"]
    return out
